# revision 7
# baseline (speedup 1.0000x reference)
"""Trainium2 Bass kernel for a diagonal selective SSM layer.

Reference computation (per batch element b):
    alpha = sigmoid(x @ Wg.T + bg)        # (L, S)
    u     = x @ WB.T + bB                 # (L, S)
    h_t   = alpha_t * h_{t-1} + u_t       # scan over L, h in R^S
    y     = h @ WC.T + bC                 # (L, D)

Sharding: data-parallel over batch. B == 8 == n_cores, so each NeuronCore
processes exactly one batch element; the small projection weights are
replicated to every core. No collectives needed.

Per-core layout (everything fp32):
  - x is fed pre-transposed as xT (D, L): contraction dim D lands on SBUF
    partitions for the two input-projection GEMMs.
  - G/U GEMMs: out (S on partitions, L free) = WgT.T @ xT, evicted from PSUM
    via ScalarE activation (Sigmoid / Identity) with the per-partition bias.
  - The recurrence runs as a single hardware linear-recurrence instruction
    per 128-channel group: nc.vector.tensor_tensor_scan (state = a*state + u
    along the free/time axis), chunk-chained via `initial`.
  - Output GEMM: y (L on partitions, D free) = h_tile.T @ WCT, so y DMAs out
    in the natural (L, D) layout; bC is added during PSUM eviction from a
    partition-broadcast bias tile.
"""

import numpy as np

B, L, D, S = 8, 2048, 1024, 256
P = 128
NCORES = 8
KD = D // P      # 8 k-tiles over the D contraction
MS = S // P      # 2 partition groups over S
NQ = 4           # L chunks for pipelining
QL = L // NQ     # 512
LT = L // P      # 16 l-tiles for the output GEMM

_NC_CACHE = {}


def _build_nc():
    import concourse.mybir as mybir
    import concourse.tile as tile
    from concourse import bacc

    f32 = mybir.dt.float32
    f32r = mybir.dt.float32r
    AF = mybir.ActivationFunctionType
    OP = mybir.AluOpType

    nc = bacc.Bacc("TRN2", target_bir_lowering=False, debug=True)

    xT = nc.dram_tensor("xT", [D, L], f32, kind="ExternalInput")
    wg = nc.dram_tensor("wgT", [D, S], f32, kind="ExternalInput")
    wb = nc.dram_tensor("wbT", [D, S], f32, kind="ExternalInput")
    wc = nc.dram_tensor("wcT", [S, D], f32, kind="ExternalInput")
    bgd = nc.dram_tensor("bg2", [S, 1], f32, kind="ExternalInput")
    bbd = nc.dram_tensor("bB2", [S, 1], f32, kind="ExternalInput")
    bcd = nc.dram_tensor("bCb", [1, D], f32, kind="ExternalInput")
    onesd = nc.dram_tensor("ones1d", [1, P], f32, kind="ExternalInput")
    y = nc.dram_tensor("y", [L, D], f32, kind="ExternalOutput")

    with tile.TileContext(nc) as tc:
        with (
            tc.tile_pool(name="persist", bufs=1) as pp,
            tc.tile_pool(name="psum", bufs=8, space="PSUM") as psp,
            tc.tile_pool(name="ystage", bufs=4) as yp,
        ):
            wgt = [pp.tile([P, S], f32r, name=f"wg{k}", tag=f"wg{k}") for k in range(KD)]
            wbt = [pp.tile([P, S], f32r, name=f"wb{k}", tag=f"wb{k}") for k in range(KD)]
            wct = [pp.tile([P, D], f32r, name=f"wc{m}", tag=f"wc{m}") for m in range(MS)]
            bgt = [pp.tile([P, 1], f32, name=f"bg{m}", tag=f"bg{m}") for m in range(MS)]
            bbt = [pp.tile([P, 1], f32, name=f"bb{m}", tag=f"bb{m}") for m in range(MS)]
            bc_row = pp.tile([1, D], f32r, name="bc_row", tag="bc_row")
            ones1 = pp.tile([1, P], f32r, name="ones1", tag="ones1")
            nc.sync.dma_start(ones1[:], onesd[:].bitcast(f32r))
            for k in range(KD):
                nc.sync.dma_start(wgt[k][:], wg[k * P:(k + 1) * P, :].bitcast(f32r))
                nc.sync.dma_start(wbt[k][:], wb[k * P:(k + 1) * P, :].bitcast(f32r))
            for m in range(MS):
                nc.sync.dma_start(wct[m][:], wc[m * P:(m + 1) * P, :].bitcast(f32r))
                nc.sync.dma_start(bgt[m][:], bgd[m * P:(m + 1) * P, :])
                nc.sync.dma_start(bbt[m][:], bbd[m * P:(m + 1) * P, :])
            nc.sync.dma_start(bc_row[:], bcd[0:1, :].bitcast(f32r))

            # x tiles, DMAed in (q, k) chunk order so the k-accumulation of
            # the first L-chunk can start before the whole 8 MB has landed.
            xs = [pp.tile([P, L], f32r, name=f"x{k}", tag=f"x{k}") for k in range(KD)]
            for q in range(NQ):
                for k in range(KD):
                    nc.sync.dma_start(
                        xs[k][:, q * QL:(q + 1) * QL],
                        xT[k * P:(k + 1) * P, q * QL:(q + 1) * QL].bitcast(f32r),
                    )

            alpha = [pp.tile([P, L], f32, name=f"al{m}", tag=f"al{m}") for m in range(MS)]
            uu = [pp.tile([P, L], f32, name=f"uu{m}", tag=f"uu{m}") for m in range(MS)]
            hh = [pp.tile([P, L], f32r, name=f"hh{m}", tag=f"hh{m}") for m in range(MS)]

            groups = [
                (wgt, bgt, alpha, AF.Sigmoid, 0),
                (wgt, bgt, alpha, AF.Sigmoid, 1),
                (wbt, bbt, uu, AF.Identity, 0),
                (wbt, bbt, uu, AF.Identity, 1),
            ]
            for q in range(NQ):
                qs = slice(q * QL, (q + 1) * QL)
                # k-outer so PE consumes x chunks as they land; 4 PSUM
                # accumulators live at once
                pss = [psp.tile([P, QL], f32, name="ps", tag="ps") for _ in groups]
                for k in range(KD):
                    for gi, (wt, bt, dst, fn, m) in enumerate(groups):
                        nc.tensor.matmul(
                            pss[gi][:],
                            wt[k][:, m * P:(m + 1) * P],
                            xs[k][:, qs],
                            start=(k == 0),
                            stop=(k == KD - 1),
                        )
                for gi, (wt, bt, dst, fn, m) in enumerate(groups):
                    nc.scalar.activation(
                        dst[m][:, qs], pss[gi][:], fn, bias=bt[m][:, 0:1], scale=1.0
                    )
                # chunk-chained hardware scan: state = alpha*state + u.
                # one channel-group per engine (DVE + GpSimd run concurrently)
                for m in range(MS):
                    init = 0.0 if q == 0 else hh[m][:, q * QL - 1:q * QL]
                    nc.vector.tensor_tensor_scan(
                        hh[m][:, qs], alpha[m][:, qs], uu[m][:, qs],
                        init, OP.mult, OP.add,
                    )
                # output GEMM for the l-tiles of this chunk; bC folded in as
                # a K=1 ones-row matmul, eviction copies split ACT/DVE
                for l in range(q * (LT // NQ), (q + 1) * (LT // NQ)):
                    ls = slice(l * P, (l + 1) * P)
                    ysb = yp.tile([P, D], f32, name="ysb", tag="ysb")
                    for nn in range(2):
                        ns = slice(nn * 512, (nn + 1) * 512)
                        ps = psp.tile([P, QL], f32, name="ps", tag="ps")
                        for m in range(MS):
                            nc.tensor.matmul(
                                ps[:],
                                hh[m][:, ls],
                                wct[m][:, ns],
                                start=(m == 0),
                                stop=False,
                            )
                        nc.tensor.matmul(
                            ps[:],
                            ones1[:],
                            bc_row[0:1, ns],
                            start=False,
                            stop=True,
                        )
                        if (l + nn) % 2 == 0:
                            nc.scalar.copy(ysb[:, ns], ps[:])
                        else:
                            nc.vector.tensor_copy(ysb[:, ns], ps[:])
                    nc.sync.dma_start(y[ls, :], ysb[:])

    nc.finalize()
    return nc


def _get_nc():
    if "nc" not in _NC_CACHE:
        _NC_CACHE["nc"] = _build_nc()
    return _NC_CACHE["nc"]


def _make_in_maps(x, Wg, bg, WB, bB, WC, bC):
    x = np.ascontiguousarray(np.asarray(x, dtype=np.float32))
    wgT = np.ascontiguousarray(np.asarray(Wg, dtype=np.float32).T)
    wbT = np.ascontiguousarray(np.asarray(WB, dtype=np.float32).T)
    wcT = np.ascontiguousarray(np.asarray(WC, dtype=np.float32).T)
    bg2 = np.ascontiguousarray(np.asarray(bg, dtype=np.float32).reshape(S, 1))
    bb2 = np.ascontiguousarray(np.asarray(bB, dtype=np.float32).reshape(S, 1))
    bcb = np.ascontiguousarray(np.asarray(bC, dtype=np.float32).reshape(1, D))
    in_maps = []
    for b in range(NCORES):
        in_maps.append({
            "xT": np.ascontiguousarray(x[b].T),
            "wgT": wgT,
            "wbT": wbT,
            "wcT": wcT,
            "bg2": bg2,
            "bB2": bb2,
            "bCb": bcb,
            "ones1d": np.ones((1, P), dtype=np.float32),
        })
    return in_maps


def _run(in_maps, **kwargs):
    from concourse.bass_utils import run_bass_kernel_spmd

    nc = _get_nc()
    return run_bass_kernel_spmd(nc, in_maps, list(range(NCORES)), **kwargs)


def kernel(x, Wg, bg, WB, bB, WC, bC):
    res = _run(_make_in_maps(x, Wg, bg, WB, bB, WC, bC))
    out = np.stack([res.results[b]["y"] for b in range(NCORES)])
    return np.ascontiguousarray(out.astype(np.float32, copy=False))


# revision 8
# speedup vs baseline: 1.6443x; 1.6443x over previous
"""Trainium2 Bass kernel for a diagonal selective SSM layer.

Reference computation (per batch element b):
    alpha = sigmoid(x @ Wg.T + bg)        # (L, S)
    u     = x @ WB.T + bB                 # (L, S)
    h_t   = alpha_t * h_{t-1} + u_t       # scan over L, h in R^S
    y     = h @ WC.T + bC                 # (L, D)

Sharding: data-parallel over batch. B == 8 == n_cores, so each NeuronCore
processes exactly one batch element; the small projection weights are
replicated to every core. No collectives needed.

Per-core dataflow (GEMM operands in float32r = single-pass full-rate fp32
matmul mode, ~2^-13 operand rounding; everything else fp32):
  - x is fed pre-transposed as xT (D, L): contraction dim D on partitions.
  - G/U GEMMs: (S on partitions, L free) = WgT.T @ xT accumulated over 8
    k-tiles in PSUM, evicted via ScalarE activation (Sigmoid / Identity)
    with the per-partition bias.
  - The recurrence is the hardware linear-recurrence instruction
    nc.vector.tensor_tensor_scan (state = a*state + u along the free/time
    axis, fp32 internal state), chunk-chained via `initial`.
  - Output GEMM: y (L on partitions, D free) = h_tile.T @ WCT, bias bC added
    during PSUM eviction from a partition-broadcast bias tile; y DMAs out in
    natural (L, D) layout.
"""

import numpy as np

B, L, D, S = 8, 2048, 1024, 256
P = 128
NCORES = 8
KD = D // P      # 8 k-tiles over the D contraction
MS = S // P      # 2 partition groups over S
NQ = 4           # L chunks for pipelining
QL = L // NQ     # 512
LT = L // P      # 16 l-tiles for the output GEMM

# experiment knobs
GU_ORDER = "wm_outer"   # "wm_outer" | "k_outer"
WARMUP_MMS = 0          # dummy matmuls to warm the PE HAM before real work

_NC_CACHE = {}


def _build_nc():
    import concourse.mybir as mybir
    import concourse.tile as tile
    from concourse import bacc

    f32 = mybir.dt.float32
    f32r = mybir.dt.float32r
    AF = mybir.ActivationFunctionType
    OP = mybir.AluOpType

    nc = bacc.Bacc("TRN2", target_bir_lowering=False, debug=True)

    xT = nc.dram_tensor("xT", [D, L], f32, kind="ExternalInput")
    wg = nc.dram_tensor("wgT", [D, S], f32, kind="ExternalInput")
    wb = nc.dram_tensor("wbT", [D, S], f32, kind="ExternalInput")
    wc = nc.dram_tensor("wcT", [S, D], f32, kind="ExternalInput")
    bgd = nc.dram_tensor("bg2", [S, 1], f32, kind="ExternalInput")
    bbd = nc.dram_tensor("bB2", [S, 1], f32, kind="ExternalInput")
    bcd = nc.dram_tensor("bCb", [P, D], f32, kind="ExternalInput")
    y = nc.dram_tensor("y", [L, D], f32, kind="ExternalOutput")

    with tile.TileContext(nc) as tc:
        with (
            tc.tile_pool(name="persist", bufs=1) as pp,
            tc.tile_pool(name="psum", bufs=8, space="PSUM") as psp,
            tc.tile_pool(name="ystage", bufs=3) as yp,
        ):
            wgt = [pp.tile([P, S], f32r, name=f"wg{k}", tag=f"wg{k}") for k in range(KD)]
            wbt = [pp.tile([P, S], f32r, name=f"wb{k}", tag=f"wb{k}") for k in range(KD)]
            wct = [pp.tile([P, D], f32r, name=f"wc{m}", tag=f"wc{m}") for m in range(MS)]
            bgt = [pp.tile([P, 1], f32, name=f"bg{m}", tag=f"bg{m}") for m in range(MS)]
            bbt = [pp.tile([P, 1], f32, name=f"bb{m}", tag=f"bb{m}") for m in range(MS)]
            bct = pp.tile([P, D], f32, name="bc", tag="bc")
            for k in range(KD):
                nc.sync.dma_start(wgt[k][:], wg[k * P:(k + 1) * P, :].bitcast(f32r))
                nc.sync.dma_start(wbt[k][:], wb[k * P:(k + 1) * P, :].bitcast(f32r))
            for m in range(MS):
                nc.sync.dma_start(wct[m][:], wc[m * P:(m + 1) * P, :].bitcast(f32r))
                nc.sync.dma_start(bgt[m][:], bgd[m * P:(m + 1) * P, :])
                nc.sync.dma_start(bbt[m][:], bbd[m * P:(m + 1) * P, :])
            nc.sync.dma_start(bct[:], bcd[:, :])

            # x tiles, DMAed in (q, k) chunk order so the k-accumulation of
            # the first L-chunk can start before the whole 8 MB has landed.
            xs = [pp.tile([P, L], f32r, name=f"x{k}", tag=f"x{k}") for k in range(KD)]
            for q in range(NQ):
                for k in range(KD):
                    nc.sync.dma_start(
                        xs[k][:, q * QL:(q + 1) * QL],
                        xT[k * P:(k + 1) * P, q * QL:(q + 1) * QL].bitcast(f32r),
                    )

            alpha = [pp.tile([P, L], f32, name=f"al{m}", tag=f"al{m}") for m in range(MS)]
            uu = [pp.tile([P, L], f32, name=f"uu{m}", tag=f"uu{m}") for m in range(MS)]
            hh = [pp.tile([P, L], f32r, name=f"hh{m}", tag=f"hh{m}") for m in range(MS)]

            if WARMUP_MMS:
                # PE HAM warm-up on already-resident weight tiles while the
                # x DMA streams in; result is discarded.
                wps = psp.tile([P, S], f32, name="wps", tag="wps")
                for i in range(WARMUP_MMS):
                    nc.tensor.matmul(
                        wps[:], wgt[0][:, 0:P], wgt[0][:, 0:S],
                        start=(i == 0), stop=(i == WARMUP_MMS - 1),
                    )

            groups = [
                (wgt, bgt, alpha, AF.Sigmoid, 0),
                (wgt, bgt, alpha, AF.Sigmoid, 1),
                (wbt, bbt, uu, AF.Identity, 0),
                (wbt, bbt, uu, AF.Identity, 1),
            ]
            for q in range(NQ):
                qs = slice(q * QL, (q + 1) * QL)
                if GU_ORDER == "wm_outer":
                    for wt, bt, dst, fn, m in groups:
                        ps = psp.tile([P, QL], f32, name="ps", tag="ps")
                        for k in range(KD):
                            nc.tensor.matmul(
                                ps[:],
                                wt[k][:, m * P:(m + 1) * P],
                                xs[k][:, qs],
                                start=(k == 0),
                                stop=(k == KD - 1),
                            )
                        nc.scalar.activation(
                            dst[m][:, qs], ps[:], fn, bias=bt[m][:, 0:1], scale=1.0
                        )
                else:  # k_outer: 4 live accumulators, PE consumes x as it lands
                    pss = [psp.tile([P, QL], f32, name="ps", tag="ps") for _ in groups]
                    for k in range(KD):
                        for gi, (wt, bt, dst, fn, m) in enumerate(groups):
                            nc.tensor.matmul(
                                pss[gi][:],
                                wt[k][:, m * P:(m + 1) * P],
                                xs[k][:, qs],
                                start=(k == 0),
                                stop=(k == KD - 1),
                            )
                    for gi, (wt, bt, dst, fn, m) in enumerate(groups):
                        nc.scalar.activation(
                            dst[m][:, qs], pss[gi][:], fn, bias=bt[m][:, 0:1], scale=1.0
                        )
                # chunk-chained hardware scan: state = alpha*state + u
                for m in range(MS):
                    init = 0.0 if q == 0 else hh[m][:, q * QL - 1:q * QL]
                    nc.vector.tensor_tensor_scan(
                        hh[m][:, qs], alpha[m][:, qs], uu[m][:, qs],
                        init, OP.mult, OP.add,
                    )
                # output GEMM for the l-tiles of this chunk
                for l in range(q * (LT // NQ), (q + 1) * (LT // NQ)):
                    ls = slice(l * P, (l + 1) * P)
                    ysb = yp.tile([P, D], f32, name="ysb", tag="ysb")
                    for nn in range(2):
                        ns = slice(nn * 512, (nn + 1) * 512)
                        ps = psp.tile([P, QL], f32, name="ps", tag="ps")
                        for m in range(MS):
                            nc.tensor.matmul(
                                ps[:],
                                hh[m][:, ls],
                                wct[m][:, ns],
                                start=(m == 0),
                                stop=(m == MS - 1),
                            )
                        nc.vector.tensor_tensor(ysb[:, ns], ps[:], bct[:, ns], OP.add)
                    nc.sync.dma_start(y[ls, :], ysb[:])

    nc.finalize()
    return nc


def _get_nc():
    if "nc" not in _NC_CACHE:
        _NC_CACHE["nc"] = _build_nc()
    return _NC_CACHE["nc"]


def _make_in_maps(x, Wg, bg, WB, bB, WC, bC):
    x = np.ascontiguousarray(np.asarray(x, dtype=np.float32))
    wgT = np.ascontiguousarray(np.asarray(Wg, dtype=np.float32).T)
    wbT = np.ascontiguousarray(np.asarray(WB, dtype=np.float32).T)
    wcT = np.ascontiguousarray(np.asarray(WC, dtype=np.float32).T)
    bg2 = np.ascontiguousarray(np.asarray(bg, dtype=np.float32).reshape(S, 1))
    bb2 = np.ascontiguousarray(np.asarray(bB, dtype=np.float32).reshape(S, 1))
    bcb = np.ascontiguousarray(
        np.broadcast_to(np.asarray(bC, dtype=np.float32).reshape(1, D), (P, D))
    )
    in_maps = []
    for b in range(NCORES):
        in_maps.append({
            "xT": np.ascontiguousarray(x[b].T),
            "wgT": wgT,
            "wbT": wbT,
            "wcT": wcT,
            "bg2": bg2,
            "bB2": bb2,
            "bCb": bcb,
        })
    return in_maps


def _run(in_maps, **kwargs):
    from concourse.bass_utils import run_bass_kernel_spmd

    nc = _get_nc()
    return run_bass_kernel_spmd(nc, in_maps, list(range(NCORES)), **kwargs)


def kernel(x, Wg, bg, WB, bB, WC, bC):
    res = _run(_make_in_maps(x, Wg, bg, WB, bB, WC, bC))
    out = np.stack([res.results[b]["y"] for b in range(NCORES)])
    return np.ascontiguousarray(out.astype(np.float32, copy=False))
